# revision 4
# baseline (speedup 1.0000x reference)
"""Trainium2 Bass kernel for grouped-top-k MoE with shared expert (8 NeuronCores, SPMD).

Strategy
--------
The reference's "dispatch" gathers rows of x by *expert id* (values 0..7), so the
routed path only ever reads x[0:8] and scatter-adds into output rows 0..7.  Writing
routed_out row i as g(w_i * x[t_i]; e_i) with t_i = chosen expert of assignment i and
e_i = ragged-segment expert of global row i, the whole routed computation factors
through a 64-row table:
    a[t,e] = x[t] @ w1[e],  b[t,e] = x[t] @ w3[e]            (tiny GEMMs)
    H[t,e] = sum_{i: t_i=t, e_i=e} silu(w_i*a[t,e]) * (w_i*b[t,e])
    delta[t] = sum_e H[t,e] @ w2[e];   out[t] += delta[t]  (t < 8)
All data-dependent indexing becomes dense one-hot algebra (exact 0/1 masks).

Sharding (8 cores):
  - data-parallel over tokens for gate + shared-expert FFN (512 tokens/core)
  - expert-parallel for w1/w3 (table build) and w2 (delta); expert c on core c
  - collectives: ONE AllGather of [tables a|b (bf16) + partial counts (bf16-exact,
    each count column <= 256)], then a ReduceScatter of H partials [64,512]->[8,512].
    A leading dummy 4-byte collective absorbs the one-time device barrier during the
    DMA prologue.

Scheduling: the routed-path chain (gate -> AG -> phi -> H -> RS -> delta) is emitted
first (= high scheduler priority); the big shared-FFN matmul stream is emitted last so
the Tile scheduler uses it to fill every PE gap, keeping the tensor engine p-state
ramped.  Stage-2 of the FFN uses hh as the stationary operand, producing token-major
output that is written bf16 (host upcasts).  Host combine: concat shard outputs, sum
the 8 per-core partial deltas, add into rows 0..7.
"""

import sys

if "/opt/trn_rl_repo" not in sys.path:
    sys.path.insert(0, "/opt/trn_rl_repo")

import numpy as np
import ml_dtypes

import concourse.bass as bass
import concourse.mybir as mybir
import concourse.tile as tile
from concourse import bacc
from concourse import bass_utils

F32 = mybir.dt.float32
BF16 = mybir.dt.bfloat16
AF = mybir.ActivationFunctionType
ALU = mybir.AluOpType
X = mybir.AxisListType.X

E = 8          # experts (== table token count == cores)
G = 4          # expert groups
D = 1024       # model dim
HID = 512      # expert hidden
SH = 1024      # shared-expert hidden
C = 8          # cores
TC = 512       # tokens per core
NTOK = 4096
BIG = 1.0e30
RG = [list(range(C))]


def ts(i, s):
    return slice(i * s, (i + 1) * s)


def build():
    nc = bacc.Bacc("TRN2", target_bir_lowering=False, debug=False, num_devices=C)

    # ---- I/O: packed partition-major [128, k, f]; contraction dim = k*128+p
    wg = nc.dram_tensor("wg", [128, 8, E], BF16, kind="ExternalInput")
    x8t = nc.dram_tensor("x8t", [128, 8, E], BF16, kind="ExternalInput")
    xtb = nc.dram_tensor("xtb", [128, 8, TC], BF16, kind="ExternalInput")
    w1c = nc.dram_tensor("w1c", [128, 8, HID], BF16, kind="ExternalInput")
    w3c = nc.dram_tensor("w3c", [128, 8, HID], BF16, kind="ExternalInput")
    w2c = nc.dram_tensor("w2c", [128, 4, D], BF16, kind="ExternalInput")
    sw1ta = nc.dram_tensor("sw1ta", [128, 8, SH // 2], BF16, kind="ExternalInput")
    sw1tb = nc.dram_tensor("sw1tb", [128, 8, SH // 2], BF16, kind="ExternalInput")
    sw3ta = nc.dram_tensor("sw3ta", [128, 8, SH // 2], BF16, kind="ExternalInput")
    sw3tb = nc.dram_tensor("sw3tb", [128, 8, SH // 2], BF16, kind="ExternalInput")
    sw2t = nc.dram_tensor("sw2t", [128, 8, D], BF16, kind="ExternalInput")
    biasd = nc.dram_tensor("biasd", [1, E], F32, kind="ExternalInput")
    niv8d = nc.dram_tensor("niv8d", [128, 8], F32, kind="ExternalInput")
    out = nc.dram_tensor("out", [TC, D], BF16, kind="ExternalOutput")   # token-major
    dout = nc.dram_tensor("dout", [E, D], F32, kind="ExternalOutput")   # partial delta

    # ---- collective bounce buffers (HBM)
    agin = nc.dram_tensor("agin", [E, 2 * HID + 4], BF16)
    agout = nc.dram_tensor("agout", [E * E, 2 * HID + 4], BF16, addr_space="Shared")
    rsin = nc.dram_tensor("rsin", [E * E, HID], BF16)
    rsout = nc.dram_tensor("rsout", [E, HID], BF16)

    # ---- compile-time constants (embedded in NEFF)
    idbf_d = nc.inline_tensor(np.eye(128, dtype=ml_dtypes.bfloat16), name="idbf")
    id8b_d = nc.inline_tensor(np.eye(E, dtype=ml_dtypes.bfloat16), name="id8b")
    # negLrep[8c+k, e] = -1 if k <= e else 0;  -offs[e] = sum_row negLrep[row,e]*cnt64[row]
    negL_np = -np.tril(np.ones((E, E), np.float32)).T
    negLrep_d = nc.inline_tensor(np.ascontiguousarray(np.tile(negL_np, (C, 1))), name="negLrep")
    ones64_d = nc.inline_tensor(np.ones((E * E, 128), np.float32), name="ones64x128")

    with tile.TileContext(nc) as tc:
        with (
            tc.tile_pool(name="wp", bufs=1) as wp,       # persistent SBUF
            tc.tile_pool(name="gp", bufs=1) as gp,       # gate/phi outputs (persist)
            tc.tile_pool(name="wk", bufs=2) as wk,       # transient SBUF
            tc.tile_pool(name="pbig", bufs=3, space="PSUM") as pbig,   # [128,1024] rotating
            tc.tile_pool(name="pm", bufs=1, space="PSUM") as pm,       # misc psum
        ):
            # ===== loads: gate deps first, FFN weights after
            wg_sb = wp.tile([128, 8, E], BF16, tag="wg")
            nc.sync.dma_start(wg_sb, wg.ap())
            x8t_sb = wp.tile([128, 8, E], BF16, tag="x8t")
            nc.sync.dma_start(x8t_sb, x8t.ap())
            xtb_sb = wp.tile([128, 8, TC], BF16, tag="xtb")
            nc.sync.dma_start(xtb_sb, xtb.ap())
            bias_sb = wp.tile([128, E], F32, tag="bias")
            nc.sync.dma_start(bias_sb, biasd.ap().to_broadcast([128, E]))
            niv8_sb = wp.tile([128, 8], F32, tag="niv8")
            nc.sync.dma_start(niv8_sb, niv8d.ap())
            negLrep_sb = wp.tile([E * E, E], F32, tag="negLrep")
            nc.sync.dma_start(negLrep_sb, negLrep_d.ap())
            ones64_sb = wp.tile([E * E, 128], F32, tag="ones64")
            nc.sync.dma_start(ones64_sb, ones64_d.ap())
            id8b_sb = wp.tile([E, E], BF16, tag="id8b")
            nc.sync.dma_start(id8b_sb, id8b_d.ap())
            w1c_sb = wp.tile([128, 8, HID], BF16, tag="w1c")
            nc.sync.dma_start(w1c_sb, w1c.ap())
            w3c_sb = wp.tile([128, 8, HID], BF16, tag="w3c")
            nc.sync.dma_start(w3c_sb, w3c.ap())
            idbf_sb = wp.tile([128, 128], BF16, tag="idbf")
            nc.sync.dma_start(idbf_sb, idbf_d.ap())
            # FFN weights (lower priority; halves so stage1 J=0..3 can start early)
            sw1_sb = wp.tile([128, 8, SH], BF16, tag="sw1")
            sw3_sb = wp.tile([128, 8, SH], BF16, tag="sw3")
            nc.sync.dma_start(sw1_sb[:, :, 0:512], sw1ta.ap())
            nc.sync.dma_start(sw3_sb[:, :, 0:512], sw3ta.ap())
            nc.sync.dma_start(sw1_sb[:, :, 512:1024], sw1tb.ap())
            nc.sync.dma_start(sw3_sb[:, :, 512:1024], sw3tb.ap())
            sw2t_sb = wp.tile([128, 8, D], BF16, tag="sw2t")
            nc.sync.dma_start(sw2t_sb, sw2t.ap())
            w2c_sb = wp.tile([128, 4, D], BF16, tag="w2c")
            nc.sync.dma_start(w2c_sb, w2c.ap())
            ones_col = wp.tile([128, 1], F32, tag="ones_col")
            nc.vector.memset(ones_col, 1.0)

            # ===== gate (f32), all 4 token-blocks fused in [128, 4, 8] ops =====
            lg4 = pm.tile([128, 4 * E], F32, tag="m1")
            for Jb in range(4):
                for kt in range(8):
                    nc.tensor.matmul(lg4[:, ts(Jb, E)],
                                     lhsT=xtb_sb[:, kt, ts(Jb, 128)],
                                     rhs=wg_sb[:, kt, :],
                                     start=(kt == 0), stop=(kt == 7))
            lg4v = lg4.rearrange("p (b e) -> p b e", e=E)

            def bc8(col):  # [128, 4] -> broadcast [128, 4, 8]
                return col.unsqueeze(2).to_broadcast([128, 4, E])

            def bc2(col16):  # [128, 4, 4] -> broadcast [128, 4, 4, 2]
                return col16.unsqueeze(3).to_broadcast([128, 4, G, 2])

            mx4 = wk.tile([128, 4], F32, tag="mx4")
            nc.vector.reduce_max(mx4, lg4v, axis=X)
            sub = wk.tile([128, 4, E], F32, tag="sub")
            nc.vector.tensor_sub(sub, lg4v, bc8(mx4))
            ex = wk.tile([128, 4, E], F32, tag="ex")
            nc.scalar.activation(ex, sub, AF.Exp)
            sm4 = wk.tile([128, 4], F32, tag="sm4")
            nc.vector.reduce_sum(sm4, ex, axis=X)
            rcp4 = wk.tile([128, 4], F32, tag="rcp4")
            nc.vector.reciprocal(rcp4, sm4)
            scores = wk.tile([128, 4, E], F32, tag="scores")
            nc.vector.tensor_mul(scores, ex, bc8(rcp4))
            s = wk.tile([128, 4, E], F32, tag="s")
            nc.vector.tensor_add(s, scores, bias_sb.unsqueeze(1).to_broadcast([128, 4, E]))
            sv = s.rearrange("p b (g two) -> p b g two", two=2)
            g4 = wk.tile([128, 4, G], F32, tag="g4")
            nc.vector.tensor_add(g4, sv[:, :, :, 0], sv[:, :, :, 1])
            gmax = wk.tile([128, 4], F32, tag="gmax")
            nc.vector.reduce_max(gmax, g4, axis=X)
            ohg1 = wk.tile([128, 4, G], F32, tag="ohg1")
            nc.vector.tensor_tensor(ohg1, g4, bc8(gmax)[:, :, 0:G], op=ALU.is_equal)
            gt = wk.tile([128, 4, G], F32, tag="gt")
            nc.vector.tensor_scalar_mul(gt, ohg1, BIG)
            g2 = wk.tile([128, 4, G], F32, tag="g2")
            nc.vector.tensor_sub(g2, g4, gt)
            gmax2 = wk.tile([128, 4], F32, tag="gmax2")
            nc.vector.reduce_max(gmax2, g2, axis=X)
            ohg2 = wk.tile([128, 4, G], F32, tag="ohg2")
            nc.vector.tensor_tensor(ohg2, g2, bc8(gmax2)[:, :, 0:G], op=ALU.is_equal)
            keep = wk.tile([128, 4, G], F32, tag="keep")
            nc.vector.tensor_add(keep, ohg1, ohg2)
            mk = wk.tile([128, 4, G], F32, tag="mk")
            nc.vector.tensor_scalar(mk, keep, BIG, BIG, op0=ALU.mult, op1=ALU.subtract)
            # masked = s*keep + (keep*BIG - BIG)   (exact select)
            m0 = wk.tile([128, 4, G, 2], F32, tag="m0")
            nc.vector.tensor_mul(m0, sv, bc2(keep))
            masked = wk.tile([128, 4, G, 2], F32, tag="masked")
            nc.vector.tensor_add(masked, m0, bc2(mk))
            maskedv = masked.rearrange("p b g two -> p b (g two)")
            m1 = wk.tile([128, 4], F32, tag="m1")
            nc.vector.reduce_max(m1, maskedv, axis=X)
            # one-hots stored interleaved [128, Jb, k, e]; weights [128, Jb, k]
            ohb = gp.tile([128, 4, 2, E], F32, tag="ohb")
            wtb = gp.tile([128, 4, 2], F32, tag="wtb")
            oh1v = ohb[:, :, 0, :]
            oh2v = ohb[:, :, 1, :]
            nc.vector.tensor_tensor(oh1v, maskedv, bc8(m1), op=ALU.is_equal)
            t2 = wk.tile([128, 4, E], F32, tag="t2")
            nc.vector.tensor_scalar_mul(t2, oh1v, BIG)
            masked2 = wk.tile([128, 4, E], F32, tag="masked2")
            nc.vector.tensor_sub(masked2, maskedv, t2)
            m2 = wk.tile([128, 4], F32, tag="m2")
            nc.vector.reduce_max(m2, masked2, axis=X)
            nc.vector.tensor_tensor(oh2v, masked2, bc8(m2), op=ALU.is_equal)
            tw1 = wk.tile([128, 4, E], F32, tag="tw1")
            nc.vector.tensor_mul(tw1, oh1v, scores)
            nc.vector.reduce_sum(wtb[:, :, 0], tw1, axis=X)
            tw2 = wk.tile([128, 4, E], F32, tag="tw2")
            nc.vector.tensor_mul(tw2, oh2v, scores)
            nc.vector.reduce_sum(wtb[:, :, 1], tw2, axis=X)

            # partial counts, bf16-exact: col j sums 2 row-sets (<=256 each)
            cnt4_ps = pm.tile([E, 4], F32, tag="m2")
            for pi in range(8):
                Jb, k = pi // 2, pi % 2
                nc.tensor.matmul(cnt4_ps[:, ts(pi // 2, 1)],
                                 lhsT=ohb[:, Jb, k, :], rhs=ones_col,
                                 start=(pi % 2 == 0), stop=(pi % 2 == 1))

            # ===== tables for expert e=core; pack [a | b | cnt4] and AllGather =====
            agin_sb = wk.tile([E, 2 * HID + 4], BF16, tag="aginsb")
            nc.scalar.copy(agin_sb[:, 2 * HID:2 * HID + 4], cnt4_ps)
            a_ps = pm.tile([E, HID], F32, tag="m2")
            for kt in range(8):
                nc.tensor.matmul(a_ps, lhsT=x8t_sb[:, kt, :], rhs=w1c_sb[:, kt, :],
                                 start=(kt == 0), stop=(kt == 7))
            nc.scalar.copy(agin_sb[:, 0:HID], a_ps)
            b_ps = pm.tile([E, HID], F32, tag="m2")
            for kt in range(8):
                nc.tensor.matmul(b_ps, lhsT=x8t_sb[:, kt, :], rhs=w3c_sb[:, kt, :],
                                 start=(kt == 0), stop=(kt == 7))
            nc.scalar.copy(agin_sb[:, HID:2 * HID], b_ps)
            nc.scalar.dma_start(agin.ap(), agin_sb)
            nc.gpsimd.collective_compute(
                "AllGather", ALU.bypass, replica_groups=RG,
                ins=[agin.ap().opt()], outs=[agout.ap().opt()],
            )

            # ===== shared-expert FFN (bf16), emitted BEFORE the phi tail: the whole FFN must precede any AG-dependent instruction in every engine queue, because the first collective cannot complete before ~70us (ncfw bootstrap) =====
            hh_sb = wp.tile([128, 8, TC], BF16, tag="hh")
            for J in range(8):
                h_ps = pbig.tile([128, 2 * TC], F32, tag="big")
                for kt in range(8):
                    nc.tensor.matmul(h_ps[:, 0:TC], lhsT=sw1_sb[:, kt, ts(J, 128)],
                                     rhs=xtb_sb[:, kt, :],
                                     start=(kt == 0), stop=(kt == 7))
                for kt in range(8):
                    nc.tensor.matmul(h_ps[:, TC:2 * TC], lhsT=sw3_sb[:, kt, ts(J, 128)],
                                     rhs=xtb_sb[:, kt, :],
                                     start=(kt == 0), stop=(kt == 7))
                sg1 = wk.tile([128, TC], F32, tag="sg1")
                nc.scalar.activation(sg1, h_ps[:, 0:TC], AF.Silu)
                nc.vector.tensor_mul(hh_sb[:, J, :], sg1, h_ps[:, TC:2 * TC])
            tabs = wp.tile([E * E, 2 * HID + 4], BF16, tag="tabs")
            nc.sync.dma_start(tabs, agout.ap())
            A_bf = tabs[:, 0:HID]
            B_bf = tabs[:, HID:2 * HID]

            # global counts -> -offsets broadcast over 128 partitions
            cnt64 = wk.tile([E * E, 1], F32, tag="cnt64")
            nc.vector.reduce_sum(cnt64, tabs[:, 2 * HID:2 * HID + 4], axis=X)
            rhs64 = wk.tile([E * E, E], F32, tag="rhs64")
            nc.vector.tensor_scalar_mul(rhs64, negLrep_sb, cnt64)
            noffs_ps = pm.tile([128, E], F32, tag="m2")
            nc.tensor.matmul(noffs_ps, lhsT=ones64_sb, rhs=rhs64, start=True, stop=True)
            noffs = wp.tile([128, E], F32, tag="noffs")
            nc.vector.tensor_copy(noffs, noffs_ps)

            # ===== phi phase: batched masks for all 8 row-sets =====
            # Gm8[p, rs, e] = (noffs[p,e] >= niv8[p,rs]) == (global_row + noffs >= 0)
            Gm8 = wk.tile([128, 8, E], F32, tag="Gm8")
            nc.vector.tensor_tensor(
                Gm8,
                noffs.unsqueeze(1).to_broadcast([128, 8, E]),
                niv8_sb.unsqueeze(2).to_broadcast([128, 8, E]),
                op=ALU.is_ge)
            osb8 = wk.tile([128, 8, E], F32, tag="osb8")
            nc.vector.tensor_sub(osb8[:, :, 1:E], Gm8[:, :, 0:E - 1], Gm8[:, :, 1:E])
            nc.vector.tensor_scalar(osb8[:, :, 0:1], Gm8[:, :, 0:1], -1.0, 1.0,
                                    op0=ALU.mult, op1=ALU.add)
            # ote_all[p, rs, e_seg, t_choice]  (0/1 exact, bf16)
            ote_all = gp.tile([128, 8, E, E], BF16, tag="ote_all")
            ohrs = ohb.rearrange("p b k e -> p (b k) e")
            nc.vector.tensor_tensor(
                ote_all,
                osb8.unsqueeze(3).to_broadcast([128, 8, E, E]),
                ohrs.unsqueeze(2).to_broadcast([128, 8, E, E]),
                op=ALU.mult)
            otev = ote_all.rearrange("p r e t -> p r (e t)")
            wtv = wtb.rearrange("p b k -> p (b k)")

            # per-rowset: transpose -> gather a/b -> phi = silu(w*a) * (w*b)
            otT_sb = wp.tile([E * E, 8, 128], BF16, tag="otT")
            phis = []
            for rs in range(8):
                otT_ps = pm.tile([E * E, 128], BF16, tag="m2")
                nc.tensor.transpose(otT_ps, otev[:, rs, :], idbf_sb)
                nc.scalar.copy(otT_sb[:, rs, :], otT_ps)
                ab_ps = pbig.tile([128, 2 * HID], F32, tag="big")
                nc.tensor.matmul(ab_ps[:, 0:HID], lhsT=otT_sb[:, rs, :], rhs=A_bf,
                                 start=True, stop=True)
                nc.tensor.matmul(ab_ps[:, HID:2 * HID], lhsT=otT_sb[:, rs, :], rhs=B_bf,
                                 start=True, stop=True)
                wtk = wtv[:, rs:rs + 1]
                sga = wk.tile([128, HID], F32, tag="sga")
                nc.scalar.activation(sga, ab_ps[:, 0:HID], AF.Silu, scale=wtk)
                phi = gp.tile([128, HID], BF16, tag=f"phi{rs}")
                nc.vector.scalar_tensor_tensor(phi, ab_ps[:, HID:2 * HID], wtk, sga,
                                               op0=ALU.mult, op1=ALU.mult)
                phis.append(phi)

            H_ps = pm.tile([E * E, HID], F32, tag="m1")
            for rs in range(8):
                nc.tensor.matmul(H_ps, lhsT=otev[:, rs, :], rhs=phis[rs],
                                 start=(rs == 0), stop=(rs == 7))
            H_sb = wk.tile([E * E, HID], BF16, tag="Hsb")
            nc.vector.tensor_copy(H_sb, H_ps)
            nc.sync.dma_start(rsin.ap(), H_sb)
            nc.gpsimd.collective_compute(
                "ReduceScatter", ALU.add, replica_groups=RG,
                ins=[rsin.ap().opt()], outs=[rsout.ap().opt()],
            )

            # stage2: out[tok, d] with hh stationary -> token-major output
            for tb in range(4):
                o_ps = pbig.tile([128, D], F32, tag="big")
                for J in range(8):
                    nc.tensor.matmul(o_ps[:, 0:512], lhsT=hh_sb[:, J, ts(tb, 128)],
                                     rhs=sw2t_sb[:, J, 0:512],
                                     start=(J == 0), stop=(J == 7))
                for J in range(8):
                    nc.tensor.matmul(o_ps[:, 512:1024], lhsT=hh_sb[:, J, ts(tb, 128)],
                                     rhs=sw2t_sb[:, J, 512:1024],
                                     start=(J == 0), stop=(J == 7))
                o_sb = wk.tile([128, D], BF16, tag="osb")
                nc.vector.tensor_copy(o_sb[:, 0:512], o_ps[:, 0:512])
                nc.scalar.copy(o_sb[:, 512:1024], o_ps[:, 512:1024])
                nc.sync.dma_start(out.ap()[ts(tb, 128), :], o_sb)



            # ===== delta for expert e=core =====
            hc = wk.tile([E, HID], BF16, tag="hc")
            nc.sync.dma_start(hc, rsout.ap())
            hct = wk.tile([128, 4 * E], BF16, tag="hct")
            hct3 = hct.rearrange("p (q e) -> p q e", q=4)
            for q in range(4):
                tp_ps = pm.tile([128, E], BF16, tag="m2")
                nc.tensor.transpose(tp_ps, hc[:, ts(q, 128)], id8b_sb)
                nc.scalar.copy(hct3[:, q, :], tp_ps)
            for n in range(2):
                d_ps = pm.tile([E, 512], F32, tag="m1")
                for q in range(4):
                    nc.tensor.matmul(d_ps, lhsT=hct3[:, q, :],
                                     rhs=w2c_sb[:, q, ts(n, 512)],
                                     start=(q == 0), stop=(q == 3))
                d_sb = wk.tile([E, 512], F32, tag="dsb")
                nc.scalar.copy(d_sb, d_ps)
                nc.sync.dma_start(dout.ap()[:, ts(n, 512)], d_sb)

    nc.compile()
    return nc


_NC = None


def _get_nc():
    global _NC
    if _NC is None:
        _NC = build()
    return _NC


def _pack(a, k):
    """[k*128, f] -> [128, k, f] partition-major contiguous."""
    kk, f = a.shape
    assert kk == k * 128
    return np.ascontiguousarray(a.reshape(k, 128, f).transpose(1, 0, 2))


def make_in_maps(x, w_gate, w1, w2, w3, sw1, sw2, sw3, expert_bias):
    bf = ml_dtypes.bfloat16
    xf = np.ascontiguousarray(np.asarray(x, np.float32).reshape(NTOK, D))
    x8t_np = _pack(np.ascontiguousarray(xf[:E].T).astype(bf), 8)
    wg_np = _pack(np.ascontiguousarray(np.asarray(w_gate, np.float32).T).astype(bf), 8)
    sw1t_np = _pack(np.ascontiguousarray(np.asarray(sw1, np.float32).T).astype(bf), 8)
    sw3t_np = _pack(np.ascontiguousarray(np.asarray(sw3, np.float32).T).astype(bf), 8)
    sw2t_np = _pack(np.ascontiguousarray(np.asarray(sw2, np.float32).T).astype(bf), 8)
    sw1ta_np = np.ascontiguousarray(sw1t_np[:, :, 0:512])
    sw1tb_np = np.ascontiguousarray(sw1t_np[:, :, 512:1024])
    sw3ta_np = np.ascontiguousarray(sw3t_np[:, :, 0:512])
    sw3tb_np = np.ascontiguousarray(sw3t_np[:, :, 512:1024])
    bias_np = np.ascontiguousarray(np.asarray(expert_bias, np.float32).reshape(1, E))
    w1_np = np.asarray(w1, np.float32)
    w2_np = np.asarray(w2, np.float32)
    w3_np = np.asarray(w3, np.float32)
    # niv8[p, rs] = -(global_row) = -(1024*c + 2*p + 256*(rs//2) + rs%2)
    rsoff = np.array([256 * (r // 2) + (r % 2) for r in range(8)], np.float32)
    p2 = 2.0 * np.arange(128, dtype=np.float32).reshape(128, 1)
    in_maps = []
    for c in range(C):
        xtT = np.ascontiguousarray(xf[c * TC:(c + 1) * TC].T)
        in_maps.append({
            "xtb": _pack(xtT.astype(bf), 8),
            "x8t": x8t_np,
            "wg": wg_np,
            "sw1ta": sw1ta_np,
            "sw1tb": sw1tb_np,
            "sw3ta": sw3ta_np,
            "sw3tb": sw3tb_np,
            "sw2t": sw2t_np,
            "w1c": _pack(np.ascontiguousarray(w1_np[c]).astype(bf), 8),
            "w3c": _pack(np.ascontiguousarray(w3_np[c]).astype(bf), 8),
            "w2c": _pack(np.ascontiguousarray(w2_np[c]).astype(bf), 4),
            "biasd": bias_np,
            "niv8d": np.ascontiguousarray(-(1024.0 * c + p2 + rsoff[None, :])),
        })
    return in_maps


def combine_outputs(results):
    full = np.empty((NTOK, D), np.float32)
    delta = np.zeros((E, D), np.float32)
    for c in range(C):
        full[c * TC:(c + 1) * TC] = results[c]["out"].astype(np.float32)
        delta += results[c]["dout"]
    full[:E] += delta
    return full.reshape(2, 2048, D)


def kernel(x, w_gate, w1, w2, w3, sw1, sw2, sw3, expert_bias, **_unused):
    nc = _get_nc()
    in_maps = make_in_maps(x, w_gate, w1, w2, w3, sw1, sw2, sw3, expert_bias)
    res = bass_utils.run_bass_kernel_spmd(nc, in_maps, core_ids=list(range(C)))
    return combine_outputs(res.results)


# revision 5
# speedup vs baseline: 1.0049x; 1.0049x over previous
"""Trainium2 Bass kernel for grouped-top-k MoE with shared expert (8 NeuronCores, SPMD).

Strategy
--------
The reference's "dispatch" gathers rows of x by *expert id* (values 0..7), so the
routed path only ever reads x[0:8] and scatter-adds into output rows 0..7.  Writing
routed_out row i as g(w_i * x[t_i]; e_i) with t_i = chosen expert of assignment i and
e_i = ragged-segment expert of global row i, the whole routed computation factors
through a 64-row table:
    a[t,e] = x[t] @ w1[e],  b[t,e] = x[t] @ w3[e]            (tiny GEMMs)
    H[t,e] = sum_{i: t_i=t, e_i=e} silu(w_i*a[t,e]) * (w_i*b[t,e])
    delta[t] = sum_e H[t,e] @ w2[e];   out[t] += delta[t]  (t < 8)
All data-dependent indexing becomes dense one-hot algebra (exact 0/1 masks).

Sharding (8 cores):
  - data-parallel over tokens for gate + shared-expert FFN (512 tokens/core)
  - expert-parallel for w1/w3 (table build) and w2 (delta); expert c on core c
  - collectives: ONE AllGather of [tables a|b (bf16) + partial counts (bf16-exact,
    each count column <= 256)], then a ReduceScatter of H partials [64,512]->[8,512].
    A leading dummy 4-byte collective absorbs the one-time device barrier during the
    DMA prologue.

Scheduling: the routed-path chain (gate -> AG -> phi -> H -> RS -> delta) is emitted
first (= high scheduler priority); the big shared-FFN matmul stream is emitted last so
the Tile scheduler uses it to fill every PE gap, keeping the tensor engine p-state
ramped.  Stage-2 of the FFN uses hh as the stationary operand, producing token-major
output that is written bf16 (host upcasts).  Host combine: concat shard outputs, sum
the 8 per-core partial deltas, add into rows 0..7.
"""

import sys

if "/opt/trn_rl_repo" not in sys.path:
    sys.path.insert(0, "/opt/trn_rl_repo")

import numpy as np
import ml_dtypes

import concourse.bass as bass
import concourse.mybir as mybir
import concourse.tile as tile
from concourse import bacc
from concourse import bass_utils

F32 = mybir.dt.float32
BF16 = mybir.dt.bfloat16
AF = mybir.ActivationFunctionType
ALU = mybir.AluOpType
X = mybir.AxisListType.X

E = 8          # experts (== table token count == cores)
G = 4          # expert groups
D = 1024       # model dim
HID = 512      # expert hidden
SH = 1024      # shared-expert hidden
C = 8          # cores
TC = 512       # tokens per core
NTOK = 4096
BIG = 1.0e30
RG = [list(range(C))]


def ts(i, s):
    return slice(i * s, (i + 1) * s)


def build():
    nc = bacc.Bacc("TRN2", target_bir_lowering=False, debug=False, num_devices=C)

    # ---- I/O: packed partition-major [128, k, f]; contraction dim = k*128+p
    wg = nc.dram_tensor("wg", [128, 8, E], BF16, kind="ExternalInput")
    x8t = nc.dram_tensor("x8t", [128, 8, E], BF16, kind="ExternalInput")
    xtb = nc.dram_tensor("xtb", [128, 8, TC], BF16, kind="ExternalInput")
    w1c = nc.dram_tensor("w1c", [128, 8, HID], BF16, kind="ExternalInput")
    w3c = nc.dram_tensor("w3c", [128, 8, HID], BF16, kind="ExternalInput")
    w2c = nc.dram_tensor("w2c", [128, 4, D], BF16, kind="ExternalInput")
    sw1ta = nc.dram_tensor("sw1ta", [128, 8, SH // 2], BF16, kind="ExternalInput")
    sw1tb = nc.dram_tensor("sw1tb", [128, 8, SH // 2], BF16, kind="ExternalInput")
    sw3ta = nc.dram_tensor("sw3ta", [128, 8, SH // 2], BF16, kind="ExternalInput")
    sw3tb = nc.dram_tensor("sw3tb", [128, 8, SH // 2], BF16, kind="ExternalInput")
    sw2t = nc.dram_tensor("sw2t", [128, 8, D], BF16, kind="ExternalInput")
    biasd = nc.dram_tensor("biasd", [1, E], F32, kind="ExternalInput")
    niv8d = nc.dram_tensor("niv8d", [128, 8], F32, kind="ExternalInput")
    out = nc.dram_tensor("out", [TC, D], BF16, kind="ExternalOutput")   # token-major
    dout = nc.dram_tensor("dout", [E, D], F32, kind="ExternalOutput")   # partial delta

    # ---- collective bounce buffers (HBM)
    agin = nc.dram_tensor("agin", [E, 2 * HID + 4], BF16)
    agout = nc.dram_tensor("agout", [E * E, 2 * HID + 4], BF16, addr_space="Shared")
    rsin = nc.dram_tensor("rsin", [E * E, HID], BF16)
    rsout = nc.dram_tensor("rsout", [E, HID], BF16)

    # ---- compile-time constants (embedded in NEFF)
    idbf_d = nc.inline_tensor(np.eye(128, dtype=ml_dtypes.bfloat16), name="idbf")
    id8b_d = nc.inline_tensor(np.eye(E, dtype=ml_dtypes.bfloat16), name="id8b")
    # negLrep[8c+k, e] = -1 if k <= e else 0;  -offs[e] = sum_row negLrep[row,e]*cnt64[row]
    negL_np = -np.tril(np.ones((E, E), np.float32)).T
    negLrep_d = nc.inline_tensor(np.ascontiguousarray(np.tile(negL_np, (C, 1))), name="negLrep")
    ones64_d = nc.inline_tensor(np.ones((E * E, 128), np.float32), name="ones64x128")

    with tile.TileContext(nc) as tc:
        with (
            tc.tile_pool(name="wp", bufs=1) as wp,       # persistent SBUF
            tc.tile_pool(name="gp", bufs=1) as gp,       # gate/phi outputs (persist)
            tc.tile_pool(name="wk", bufs=2) as wk,       # transient SBUF
            tc.tile_pool(name="pbig", bufs=3, space="PSUM") as pbig,   # [128,1024] rotating
            tc.tile_pool(name="pm", bufs=1, space="PSUM") as pm,       # misc psum
        ):
            # ===== loads: gate deps first, FFN weights after
            wg_sb = wp.tile([128, 8, E], BF16, tag="wg")
            nc.sync.dma_start(wg_sb, wg.ap())
            x8t_sb = wp.tile([128, 8, E], BF16, tag="x8t")
            nc.sync.dma_start(x8t_sb, x8t.ap())
            xtb_sb = wp.tile([128, 8, TC], BF16, tag="xtb")
            nc.sync.dma_start(xtb_sb, xtb.ap())
            bias_sb = wp.tile([128, E], F32, tag="bias")
            nc.sync.dma_start(bias_sb, biasd.ap().to_broadcast([128, E]))
            niv8_sb = wp.tile([128, 8], F32, tag="niv8")
            nc.sync.dma_start(niv8_sb, niv8d.ap())
            negLrep_sb = wp.tile([E * E, E], F32, tag="negLrep")
            nc.sync.dma_start(negLrep_sb, negLrep_d.ap())
            ones64_sb = wp.tile([E * E, 128], F32, tag="ones64")
            nc.sync.dma_start(ones64_sb, ones64_d.ap())
            id8b_sb = wp.tile([E, E], BF16, tag="id8b")
            nc.sync.dma_start(id8b_sb, id8b_d.ap())
            w1c_sb = wp.tile([128, 8, HID], BF16, tag="w1c")
            nc.sync.dma_start(w1c_sb, w1c.ap())
            w3c_sb = wp.tile([128, 8, HID], BF16, tag="w3c")
            nc.sync.dma_start(w3c_sb, w3c.ap())
            idbf_sb = wp.tile([128, 128], BF16, tag="idbf")
            nc.sync.dma_start(idbf_sb, idbf_d.ap())
            # FFN weights (lower priority; halves so stage1 J=0..3 can start early)
            sw1_sb = wp.tile([128, 8, SH], BF16, tag="sw1")
            sw3_sb = wp.tile([128, 8, SH], BF16, tag="sw3")
            nc.sync.dma_start(sw1_sb[:, :, 0:512], sw1ta.ap())
            nc.sync.dma_start(sw3_sb[:, :, 0:512], sw3ta.ap())
            nc.sync.dma_start(sw1_sb[:, :, 512:1024], sw1tb.ap())
            nc.sync.dma_start(sw3_sb[:, :, 512:1024], sw3tb.ap())
            sw2t_sb = wp.tile([128, 8, D], BF16, tag="sw2t")
            nc.sync.dma_start(sw2t_sb, sw2t.ap())
            w2c_sb = wp.tile([128, 4, D], BF16, tag="w2c")
            nc.sync.dma_start(w2c_sb, w2c.ap())
            ones_col = wp.tile([128, 1], F32, tag="ones_col")
            nc.vector.memset(ones_col, 1.0)

            # ===== gate (f32), all 4 token-blocks fused in [128, 4, 8] ops =====
            lg4 = pm.tile([128, 4 * E], F32, tag="m1")
            for Jb in range(4):
                for kt in range(8):
                    nc.tensor.matmul(lg4[:, ts(Jb, E)],
                                     lhsT=xtb_sb[:, kt, ts(Jb, 128)],
                                     rhs=wg_sb[:, kt, :],
                                     start=(kt == 0), stop=(kt == 7))
            lg4v = lg4.rearrange("p (b e) -> p b e", e=E)

            def bc8(col):  # [128, 4] -> broadcast [128, 4, 8]
                return col.unsqueeze(2).to_broadcast([128, 4, E])

            def bc2(col16):  # [128, 4, 4] -> broadcast [128, 4, 4, 2]
                return col16.unsqueeze(3).to_broadcast([128, 4, G, 2])

            mx4 = wk.tile([128, 4], F32, tag="mx4")
            nc.vector.reduce_max(mx4, lg4v, axis=X)
            sub = wk.tile([128, 4, E], F32, tag="sub")
            nc.vector.tensor_sub(sub, lg4v, bc8(mx4))
            ex = wk.tile([128, 4, E], F32, tag="ex")
            nc.scalar.activation(ex, sub, AF.Exp)
            sm4 = wk.tile([128, 4], F32, tag="sm4")
            nc.vector.reduce_sum(sm4, ex, axis=X)
            rcp4 = wk.tile([128, 4], F32, tag="rcp4")
            nc.vector.reciprocal(rcp4, sm4)
            scores = wk.tile([128, 4, E], F32, tag="scores")
            nc.vector.tensor_mul(scores, ex, bc8(rcp4))
            s = wk.tile([128, 4, E], F32, tag="s")
            nc.vector.tensor_add(s, scores, bias_sb.unsqueeze(1).to_broadcast([128, 4, E]))
            sv = s.rearrange("p b (g two) -> p b g two", two=2)
            g4 = wk.tile([128, 4, G], F32, tag="g4")
            nc.vector.tensor_add(g4, sv[:, :, :, 0], sv[:, :, :, 1])
            gmax = wk.tile([128, 4], F32, tag="gmax")
            nc.vector.reduce_max(gmax, g4, axis=X)
            ohg1 = wk.tile([128, 4, G], F32, tag="ohg1")
            nc.vector.tensor_tensor(ohg1, g4, bc8(gmax)[:, :, 0:G], op=ALU.is_equal)
            gt = wk.tile([128, 4, G], F32, tag="gt")
            nc.vector.tensor_scalar_mul(gt, ohg1, BIG)
            g2 = wk.tile([128, 4, G], F32, tag="g2")
            nc.vector.tensor_sub(g2, g4, gt)
            gmax2 = wk.tile([128, 4], F32, tag="gmax2")
            nc.vector.reduce_max(gmax2, g2, axis=X)
            ohg2 = wk.tile([128, 4, G], F32, tag="ohg2")
            nc.vector.tensor_tensor(ohg2, g2, bc8(gmax2)[:, :, 0:G], op=ALU.is_equal)
            keep = wk.tile([128, 4, G], F32, tag="keep")
            nc.vector.tensor_add(keep, ohg1, ohg2)
            mk = wk.tile([128, 4, G], F32, tag="mk")
            nc.vector.tensor_scalar(mk, keep, BIG, BIG, op0=ALU.mult, op1=ALU.subtract)
            # masked = s*keep + (keep*BIG - BIG)   (exact select)
            m0 = wk.tile([128, 4, G, 2], F32, tag="m0")
            nc.vector.tensor_mul(m0, sv, bc2(keep))
            masked = wk.tile([128, 4, G, 2], F32, tag="masked")
            nc.vector.tensor_add(masked, m0, bc2(mk))
            maskedv = masked.rearrange("p b g two -> p b (g two)")
            m1 = wk.tile([128, 4], F32, tag="m1")
            nc.vector.reduce_max(m1, maskedv, axis=X)
            # one-hots stored interleaved [128, Jb, k, e]; weights [128, Jb, k]
            ohb = gp.tile([128, 4, 2, E], F32, tag="ohb")
            wtb = gp.tile([128, 4, 2], F32, tag="wtb")
            oh1v = ohb[:, :, 0, :]
            oh2v = ohb[:, :, 1, :]
            nc.vector.tensor_tensor(oh1v, maskedv, bc8(m1), op=ALU.is_equal)
            t2 = wk.tile([128, 4, E], F32, tag="t2")
            nc.vector.tensor_scalar_mul(t2, oh1v, BIG)
            masked2 = wk.tile([128, 4, E], F32, tag="masked2")
            nc.vector.tensor_sub(masked2, maskedv, t2)
            m2 = wk.tile([128, 4], F32, tag="m2")
            nc.vector.reduce_max(m2, masked2, axis=X)
            nc.vector.tensor_tensor(oh2v, masked2, bc8(m2), op=ALU.is_equal)
            tw1 = wk.tile([128, 4, E], F32, tag="tw1")
            nc.vector.tensor_mul(tw1, oh1v, scores)
            nc.vector.reduce_sum(wtb[:, :, 0], tw1, axis=X)
            tw2 = wk.tile([128, 4, E], F32, tag="tw2")
            nc.vector.tensor_mul(tw2, oh2v, scores)
            nc.vector.reduce_sum(wtb[:, :, 1], tw2, axis=X)

            # partial counts, bf16-exact: col j sums 2 row-sets (<=256 each)
            cnt4_ps = pm.tile([E, 4], F32, tag="m2")
            for pi in range(8):
                Jb, k = pi // 2, pi % 2
                nc.tensor.matmul(cnt4_ps[:, ts(pi // 2, 1)],
                                 lhsT=ohb[:, Jb, k, :], rhs=ones_col,
                                 start=(pi % 2 == 0), stop=(pi % 2 == 1))

            # ===== tables for expert e=core; pack [a | b | cnt4] and AllGather =====
            agin_sb = wk.tile([E, 2 * HID + 4], BF16, tag="aginsb")
            nc.scalar.copy(agin_sb[:, 2 * HID:2 * HID + 4], cnt4_ps)
            a_ps = pm.tile([E, HID], F32, tag="m2")
            for kt in range(8):
                nc.tensor.matmul(a_ps, lhsT=x8t_sb[:, kt, :], rhs=w1c_sb[:, kt, :],
                                 start=(kt == 0), stop=(kt == 7))
            nc.scalar.copy(agin_sb[:, 0:HID], a_ps)
            b_ps = pm.tile([E, HID], F32, tag="m2")
            for kt in range(8):
                nc.tensor.matmul(b_ps, lhsT=x8t_sb[:, kt, :], rhs=w3c_sb[:, kt, :],
                                 start=(kt == 0), stop=(kt == 7))
            nc.scalar.copy(agin_sb[:, HID:2 * HID], b_ps)
            nc.scalar.dma_start(agin.ap(), agin_sb)
            nc.gpsimd.collective_compute(
                "AllGather", ALU.bypass, replica_groups=RG,
                ins=[agin.ap().opt()], outs=[agout.ap().opt()],
            )

            # ===== shared-expert FFN (bf16), emitted BEFORE the phi tail: the whole FFN must precede any AG-dependent instruction in every engine queue, because the first collective cannot complete before ~70us (ncfw bootstrap) =====
            hh_sb = wp.tile([128, 8, TC], BF16, tag="hh")
            for J in range(8):
                h_ps = pbig.tile([128, 2 * TC], F32, tag="big")
                for kt in range(8):
                    nc.tensor.matmul(h_ps[:, 0:TC], lhsT=sw1_sb[:, kt, ts(J, 128)],
                                     rhs=xtb_sb[:, kt, :],
                                     start=(kt == 0), stop=(kt == 7))
                for kt in range(8):
                    nc.tensor.matmul(h_ps[:, TC:2 * TC], lhsT=sw3_sb[:, kt, ts(J, 128)],
                                     rhs=xtb_sb[:, kt, :],
                                     start=(kt == 0), stop=(kt == 7))
                sg1 = wk.tile([128, TC], F32, tag="sg1")
                nc.scalar.activation(sg1, h_ps[:, 0:TC], AF.Silu)
                nc.vector.tensor_mul(hh_sb[:, J, :], sg1, h_ps[:, TC:2 * TC])
            # stage2: out[tok, d] with hh stationary -> token-major output
            for tb in range(0, 2):
                o_ps = pbig.tile([128, D], F32, tag="big")
                for J in range(8):
                    nc.tensor.matmul(o_ps[:, 0:512], lhsT=hh_sb[:, J, ts(tb, 128)],
                                     rhs=sw2t_sb[:, J, 0:512],
                                     start=(J == 0), stop=(J == 7))
                for J in range(8):
                    nc.tensor.matmul(o_ps[:, 512:1024], lhsT=hh_sb[:, J, ts(tb, 128)],
                                     rhs=sw2t_sb[:, J, 512:1024],
                                     start=(J == 0), stop=(J == 7))
                o_sb = wk.tile([128, D], BF16, tag="osb")
                nc.vector.tensor_copy(o_sb[:, 0:512], o_ps[:, 0:512])
                nc.scalar.copy(o_sb[:, 512:1024], o_ps[:, 512:1024])
                nc.sync.dma_start(out.ap()[ts(tb, 128), :], o_sb)




            tabs = wp.tile([E * E, 2 * HID + 4], BF16, tag="tabs")
            nc.sync.dma_start(tabs, agout.ap())
            A_bf = tabs[:, 0:HID]
            B_bf = tabs[:, HID:2 * HID]

            # global counts -> -offsets broadcast over 128 partitions
            cnt64 = wk.tile([E * E, 1], F32, tag="cnt64")
            nc.vector.reduce_sum(cnt64, tabs[:, 2 * HID:2 * HID + 4], axis=X)
            rhs64 = wk.tile([E * E, E], F32, tag="rhs64")
            nc.vector.tensor_scalar_mul(rhs64, negLrep_sb, cnt64)
            noffs_ps = pm.tile([128, E], F32, tag="m2")
            nc.tensor.matmul(noffs_ps, lhsT=ones64_sb, rhs=rhs64, start=True, stop=True)
            noffs = wp.tile([128, E], F32, tag="noffs")
            nc.vector.tensor_copy(noffs, noffs_ps)

            # ===== phi phase: batched masks for all 8 row-sets =====
            # Gm8[p, rs, e] = (noffs[p,e] >= niv8[p,rs]) == (global_row + noffs >= 0)
            Gm8 = wk.tile([128, 8, E], F32, tag="Gm8")
            nc.vector.tensor_tensor(
                Gm8,
                noffs.unsqueeze(1).to_broadcast([128, 8, E]),
                niv8_sb.unsqueeze(2).to_broadcast([128, 8, E]),
                op=ALU.is_ge)
            osb8 = wk.tile([128, 8, E], F32, tag="osb8")
            nc.vector.tensor_sub(osb8[:, :, 1:E], Gm8[:, :, 0:E - 1], Gm8[:, :, 1:E])
            nc.vector.tensor_scalar(osb8[:, :, 0:1], Gm8[:, :, 0:1], -1.0, 1.0,
                                    op0=ALU.mult, op1=ALU.add)
            # ote_all[p, rs, e_seg, t_choice]  (0/1 exact, bf16)
            ote_all = gp.tile([128, 8, E, E], BF16, tag="ote_all")
            ohrs = ohb.rearrange("p b k e -> p (b k) e")
            nc.vector.tensor_tensor(
                ote_all,
                osb8.unsqueeze(3).to_broadcast([128, 8, E, E]),
                ohrs.unsqueeze(2).to_broadcast([128, 8, E, E]),
                op=ALU.mult)
            otev = ote_all.rearrange("p r e t -> p r (e t)")
            wtv = wtb.rearrange("p b k -> p (b k)")

            # per-rowset: transpose -> gather a/b -> phi = silu(w*a) * (w*b)
            otT_sb = wp.tile([E * E, 8, 128], BF16, tag="otT")
            phis = []
            for rs in range(8):
                otT_ps = pm.tile([E * E, 128], BF16, tag="m2")
                nc.tensor.transpose(otT_ps, otev[:, rs, :], idbf_sb)
                nc.scalar.copy(otT_sb[:, rs, :], otT_ps)
                ab_ps = pbig.tile([128, 2 * HID], F32, tag="big")
                nc.tensor.matmul(ab_ps[:, 0:HID], lhsT=otT_sb[:, rs, :], rhs=A_bf,
                                 start=True, stop=True)
                nc.tensor.matmul(ab_ps[:, HID:2 * HID], lhsT=otT_sb[:, rs, :], rhs=B_bf,
                                 start=True, stop=True)
                wtk = wtv[:, rs:rs + 1]
                sga = wk.tile([128, HID], F32, tag="sga")
                nc.scalar.activation(sga, ab_ps[:, 0:HID], AF.Silu, scale=wtk)
                phi = gp.tile([128, HID], BF16, tag=f"phi{rs}")
                nc.vector.scalar_tensor_tensor(phi, ab_ps[:, HID:2 * HID], wtk, sga,
                                               op0=ALU.mult, op1=ALU.mult)
                phis.append(phi)

            H_ps = pm.tile([E * E, HID], F32, tag="m1")
            for rs in range(8):
                nc.tensor.matmul(H_ps, lhsT=otev[:, rs, :], rhs=phis[rs],
                                 start=(rs == 0), stop=(rs == 7))
            H_sb = wk.tile([E * E, HID], BF16, tag="Hsb")
            nc.vector.tensor_copy(H_sb, H_ps)
            nc.sync.dma_start(rsin.ap(), H_sb)
            nc.gpsimd.collective_compute(
                "ReduceScatter", ALU.add, replica_groups=RG,
                ins=[rsin.ap().opt()], outs=[rsout.ap().opt()],
            )

            # stage2 second half (after the phi block in every engine queue)
            for tb in range(2, 4):
                o_ps = pbig.tile([128, D], F32, tag="big")
                for J in range(8):
                    nc.tensor.matmul(o_ps[:, 0:512], lhsT=hh_sb[:, J, ts(tb, 128)],
                                     rhs=sw2t_sb[:, J, 0:512],
                                     start=(J == 0), stop=(J == 7))
                for J in range(8):
                    nc.tensor.matmul(o_ps[:, 512:1024], lhsT=hh_sb[:, J, ts(tb, 128)],
                                     rhs=sw2t_sb[:, J, 512:1024],
                                     start=(J == 0), stop=(J == 7))
                o_sb = wk.tile([128, D], BF16, tag="osb")
                nc.vector.tensor_copy(o_sb[:, 0:512], o_ps[:, 0:512])
                nc.scalar.copy(o_sb[:, 512:1024], o_ps[:, 512:1024])
                nc.sync.dma_start(out.ap()[ts(tb, 128), :], o_sb)




            # ===== delta for expert e=core =====
            hc = wk.tile([E, HID], BF16, tag="hc")
            nc.sync.dma_start(hc, rsout.ap())
            hct = wk.tile([128, 4 * E], BF16, tag="hct")
            hct3 = hct.rearrange("p (q e) -> p q e", q=4)
            for q in range(4):
                tp_ps = pm.tile([128, E], BF16, tag="m2")
                nc.tensor.transpose(tp_ps, hc[:, ts(q, 128)], id8b_sb)
                nc.scalar.copy(hct3[:, q, :], tp_ps)
            for n in range(2):
                d_ps = pm.tile([E, 512], F32, tag="m1")
                for q in range(4):
                    nc.tensor.matmul(d_ps, lhsT=hct3[:, q, :],
                                     rhs=w2c_sb[:, q, ts(n, 512)],
                                     start=(q == 0), stop=(q == 3))
                d_sb = wk.tile([E, 512], F32, tag="dsb")
                nc.scalar.copy(d_sb, d_ps)
                nc.sync.dma_start(dout.ap()[:, ts(n, 512)], d_sb)

    nc.compile()
    return nc


_NC = None


def _get_nc():
    global _NC
    if _NC is None:
        _NC = build()
    return _NC


def _pack(a, k):
    """[k*128, f] -> [128, k, f] partition-major contiguous."""
    kk, f = a.shape
    assert kk == k * 128
    return np.ascontiguousarray(a.reshape(k, 128, f).transpose(1, 0, 2))


def make_in_maps(x, w_gate, w1, w2, w3, sw1, sw2, sw3, expert_bias):
    bf = ml_dtypes.bfloat16
    xf = np.ascontiguousarray(np.asarray(x, np.float32).reshape(NTOK, D))
    x8t_np = _pack(np.ascontiguousarray(xf[:E].T).astype(bf), 8)
    wg_np = _pack(np.ascontiguousarray(np.asarray(w_gate, np.float32).T).astype(bf), 8)
    sw1t_np = _pack(np.ascontiguousarray(np.asarray(sw1, np.float32).T).astype(bf), 8)
    sw3t_np = _pack(np.ascontiguousarray(np.asarray(sw3, np.float32).T).astype(bf), 8)
    sw2t_np = _pack(np.ascontiguousarray(np.asarray(sw2, np.float32).T).astype(bf), 8)
    sw1ta_np = np.ascontiguousarray(sw1t_np[:, :, 0:512])
    sw1tb_np = np.ascontiguousarray(sw1t_np[:, :, 512:1024])
    sw3ta_np = np.ascontiguousarray(sw3t_np[:, :, 0:512])
    sw3tb_np = np.ascontiguousarray(sw3t_np[:, :, 512:1024])
    bias_np = np.ascontiguousarray(np.asarray(expert_bias, np.float32).reshape(1, E))
    w1_np = np.asarray(w1, np.float32)
    w2_np = np.asarray(w2, np.float32)
    w3_np = np.asarray(w3, np.float32)
    # niv8[p, rs] = -(global_row) = -(1024*c + 2*p + 256*(rs//2) + rs%2)
    rsoff = np.array([256 * (r // 2) + (r % 2) for r in range(8)], np.float32)
    p2 = 2.0 * np.arange(128, dtype=np.float32).reshape(128, 1)
    in_maps = []
    for c in range(C):
        xtT = np.ascontiguousarray(xf[c * TC:(c + 1) * TC].T)
        in_maps.append({
            "xtb": _pack(xtT.astype(bf), 8),
            "x8t": x8t_np,
            "wg": wg_np,
            "sw1ta": sw1ta_np,
            "sw1tb": sw1tb_np,
            "sw3ta": sw3ta_np,
            "sw3tb": sw3tb_np,
            "sw2t": sw2t_np,
            "w1c": _pack(np.ascontiguousarray(w1_np[c]).astype(bf), 8),
            "w3c": _pack(np.ascontiguousarray(w3_np[c]).astype(bf), 8),
            "w2c": _pack(np.ascontiguousarray(w2_np[c]).astype(bf), 4),
            "biasd": bias_np,
            "niv8d": np.ascontiguousarray(-(1024.0 * c + p2 + rsoff[None, :])),
        })
    return in_maps


def combine_outputs(results):
    full = np.empty((NTOK, D), np.float32)
    delta = np.zeros((E, D), np.float32)
    for c in range(C):
        full[c * TC:(c + 1) * TC] = results[c]["out"].astype(np.float32)
        delta += results[c]["dout"]
    full[:E] += delta
    return full.reshape(2, 2048, D)


def kernel(x, w_gate, w1, w2, w3, sw1, sw2, sw3, expert_bias, **_unused):
    nc = _get_nc()
    in_maps = make_in_maps(x, w_gate, w1, w2, w3, sw1, sw2, sw3, expert_bias)
    res = bass_utils.run_bass_kernel_spmd(nc, in_maps, core_ids=list(range(C)))
    return combine_outputs(res.results)


# revision 13
# speedup vs baseline: 1.1210x; 1.1156x over previous
"""Trainium2 Bass kernel for grouped-top-k MoE with shared expert (8 NeuronCores, SPMD).

Strategy
--------
The reference's "dispatch" gathers rows of x by *expert id* (values 0..7), so the
routed path only ever reads x[0:8] and scatter-adds into output rows 0..7.  Writing
routed_out row i as g(w_i * x[t_i]; e_i) with t_i = chosen expert of assignment i and
e_i = ragged-segment expert of global row i, the whole routed computation factors
through a 64-row table:
    a[t,e] = x[t] @ w1[e],  b[t,e] = x[t] @ w3[e]            (tiny GEMMs)
    H[t,e] = sum_{i: t_i=t, e_i=e} silu(w_i*a[t,e]) * (w_i*b[t,e])
    delta[t] = sum_e H[t,e] @ w2[e];   out[t] += delta[t]  (t < 8)
All data-dependent indexing becomes dense one-hot algebra (exact 0/1 masks).

Sharding (8 cores):
  - data-parallel over tokens for gate + shared-expert FFN (512 tokens/core)
  - expert-parallel for w1/w3 (table build) and w2 (delta); expert c on core c
  - collectives: ONE AllGather of [tables a|b (bf16) + partial counts packed
    bf16-exact as four <=256 columns], then one bf16 ReduceScatter of H partials
    [64,512] -> [8,512].

Performance notes (measured on trn2):
  - The first collective of a NEFF execution cannot complete before ~70-90us: the
    device BARRIER (ncfw bootstrap + cross-core launch skew) ends only ~5-30us
    after the last core's doorbell, and the collective itself starts ~11us later.
    The kernel is therefore structured so NOTHING on any engine queue depends on
    the AllGather until all FFN work has been emitted: gate -> counts/tables ->
    AG trigger early, then the whole shared FFN, then the AG-dependent phi/H/RS/
    delta tail.  tc.tile_wait_until() pins the tail's sim-readiness late so the
    TileScheduler (whose collective cost model assumes 15us) cannot interleave
    tail ops ahead of FFN work in the static engine orders.
  - The agin/rsin DMAs are issued from the Act/GpSimd queues: SP-issued DMAs get
    their completion semaphore bucketed with unrelated multi-MB weight loads,
    which was observed to delay the collective doorbells by 10-20us.
  - The tensor engine is utilization-throttled (~60%): sustained bf16 matmul
    streams run at ~435ns per 512-col instruction, so the 192-matmul FFN is
    ~80us of PE time; fp8 would halve it but fails the 2e-2 accuracy gate
    (measured 5-6% rel err for an fp8 shared FFN, 2.0% for fp8 tables).
  - Stage-2 of the FFN uses hh as the stationary operand, producing token-major
    output written bf16 (host upcasts; no host-side transpose).

Host combine: concat shard outputs, sum the 8 per-core partial deltas, add into
rows 0..7.  Typical HW exec time ~136-152us (run-to-run variance is dominated by
the collective-barrier rendezvous, which pays per-core NEFF launch skew).
"""

import sys

if "/opt/trn_rl_repo" not in sys.path:
    sys.path.insert(0, "/opt/trn_rl_repo")

import numpy as np
import ml_dtypes

import concourse.bass as bass
import concourse.mybir as mybir
import concourse.tile as tile
from concourse import bacc
from concourse import bass_utils

F32 = mybir.dt.float32
BF16 = mybir.dt.bfloat16
AF = mybir.ActivationFunctionType
ALU = mybir.AluOpType
X = mybir.AxisListType.X

E = 8          # experts (== table token count == cores)
G = 4          # expert groups
D = 1024       # model dim
HID = 512      # expert hidden
SH = 1024      # shared-expert hidden
C = 8          # cores
TC = 512       # tokens per core
NTOK = 4096
BIG = 1.0e30
RG = [list(range(C))]


def ts(i, s):
    return slice(i * s, (i + 1) * s)


def build():
    nc = bacc.Bacc("TRN2", target_bir_lowering=False, debug=False, num_devices=C)

    # ---- I/O: packed partition-major [128, k, f]; contraction dim = k*128+p
    wg = nc.dram_tensor("wg", [128, 8, E], BF16, kind="ExternalInput")
    x8t = nc.dram_tensor("x8t", [128, 8, E], BF16, kind="ExternalInput")
    xtb = nc.dram_tensor("xtb", [128, 8, TC], BF16, kind="ExternalInput")
    w1c = nc.dram_tensor("w1c", [128, 8, HID], BF16, kind="ExternalInput")
    w3c = nc.dram_tensor("w3c", [128, 8, HID], BF16, kind="ExternalInput")
    w2c = nc.dram_tensor("w2c", [128, 4, D], BF16, kind="ExternalInput")
    sw1ta = nc.dram_tensor("sw1ta", [128, 8, SH // 2], BF16, kind="ExternalInput")
    sw1tb = nc.dram_tensor("sw1tb", [128, 8, SH // 2], BF16, kind="ExternalInput")
    sw3ta = nc.dram_tensor("sw3ta", [128, 8, SH // 2], BF16, kind="ExternalInput")
    sw3tb = nc.dram_tensor("sw3tb", [128, 8, SH // 2], BF16, kind="ExternalInput")
    sw2t = nc.dram_tensor("sw2t", [128, 8, D], BF16, kind="ExternalInput")
    biasd = nc.dram_tensor("biasd", [1, E], F32, kind="ExternalInput")
    niv8d = nc.dram_tensor("niv8d", [128, 8], F32, kind="ExternalInput")
    out = nc.dram_tensor("out", [TC, D], BF16, kind="ExternalOutput")   # token-major
    dout = nc.dram_tensor("dout", [E, D], F32, kind="ExternalOutput")   # partial delta

    # ---- collective bounce buffers (HBM)
    agin = nc.dram_tensor("agin", [E, 2 * HID + 4], BF16)
    agout = nc.dram_tensor("agout", [E * E, 2 * HID + 4], BF16, addr_space="Shared")
    rsin = nc.dram_tensor("rsin", [E * E, HID], BF16)
    rsout = nc.dram_tensor("rsout", [E, HID], BF16)

    # ---- compile-time constants (embedded in NEFF)
    idbf_d = nc.inline_tensor(np.eye(128, dtype=ml_dtypes.bfloat16), name="idbf")
    id8b_d = nc.inline_tensor(np.eye(E, dtype=ml_dtypes.bfloat16), name="id8b")
    # negLrep[8c+k, e] = -1 if k <= e else 0;  -offs[e] = sum_row negLrep[row,e]*cnt64[row]
    negL_np = -np.tril(np.ones((E, E), np.float32)).T
    negLrep_d = nc.inline_tensor(np.ascontiguousarray(np.tile(negL_np, (C, 1))), name="negLrep")
    ones64_d = nc.inline_tensor(np.ones((E * E, 128), np.float32), name="ones64x128")

    with tile.TileContext(nc) as tc:
        with (
            tc.tile_pool(name="wp", bufs=1) as wp,       # persistent SBUF
            tc.tile_pool(name="gp", bufs=1) as gp,       # gate/phi outputs (persist)
            tc.tile_pool(name="wk", bufs=2) as wk,       # transient SBUF
            tc.tile_pool(name="pbig", bufs=2, space="PSUM") as pbig,
            tc.tile_pool(name="pot", bufs=2, space="PSUM") as pot,   # [128,1024] rotating
            tc.tile_pool(name="pm", bufs=1, space="PSUM") as pm,       # misc psum
        ):
            # ===== loads: gate deps first, FFN weights after
            wg_sb = wp.tile([128, 8, E], BF16, tag="wg")
            nc.sync.dma_start(wg_sb, wg.ap())
            x8t_sb = wp.tile([128, 8, E], BF16, tag="x8t")
            nc.sync.dma_start(x8t_sb, x8t.ap())
            xtb_sb = wp.tile([128, 8, TC], BF16, tag="xtb")
            nc.sync.dma_start(xtb_sb, xtb.ap())
            bias_sb = wp.tile([128, E], F32, tag="bias")
            nc.sync.dma_start(bias_sb, biasd.ap().to_broadcast([128, E]))
            niv8_sb = wp.tile([128, 8], F32, tag="niv8")
            nc.sync.dma_start(niv8_sb, niv8d.ap())
            negLrep_sb = wp.tile([E * E, E], F32, tag="negLrep")
            nc.sync.dma_start(negLrep_sb, negLrep_d.ap())
            ones64_sb = wp.tile([E * E, 128], F32, tag="ones64")
            nc.sync.dma_start(ones64_sb, ones64_d.ap())
            id8b_sb = wp.tile([E, E], BF16, tag="id8b")
            nc.sync.dma_start(id8b_sb, id8b_d.ap())
            w1c_sb = wp.tile([128, 8, HID], BF16, tag="w1c")
            nc.sync.dma_start(w1c_sb, w1c.ap())
            w3c_sb = wp.tile([128, 8, HID], BF16, tag="w3c")
            nc.sync.dma_start(w3c_sb, w3c.ap())
            idbf_sb = wp.tile([128, 128], BF16, tag="idbf")
            nc.sync.dma_start(idbf_sb, idbf_d.ap())
            # FFN weights (lower priority; halves so stage1 J=0..3 can start early)
            sw1_sb = wp.tile([128, 8, SH], BF16, tag="sw1")
            sw3_sb = wp.tile([128, 8, SH], BF16, tag="sw3")
            nc.sync.dma_start(sw1_sb[:, :, 0:512], sw1ta.ap())
            nc.sync.dma_start(sw3_sb[:, :, 0:512], sw3ta.ap())
            nc.sync.dma_start(sw1_sb[:, :, 512:1024], sw1tb.ap())
            nc.sync.dma_start(sw3_sb[:, :, 512:1024], sw3tb.ap())
            sw2t_sb = wp.tile([128, 8, D], BF16, tag="sw2t")
            nc.sync.dma_start(sw2t_sb, sw2t.ap())
            w2c_sb = wp.tile([128, 4, D], BF16, tag="w2c")
            nc.sync.dma_start(w2c_sb, w2c.ap())
            ones_col = wp.tile([128, 1], F32, tag="ones_col")
            nc.vector.memset(ones_col, 1.0)

            # ===== gate (f32), all 4 token-blocks fused in [128, 4, 8] ops =====
            lg4 = pm.tile([128, 4 * E], F32, tag="m1")
            for Jb in range(4):
                for kt in range(8):
                    nc.tensor.matmul(lg4[:, ts(Jb, E)],
                                     lhsT=xtb_sb[:, kt, ts(Jb, 128)],
                                     rhs=wg_sb[:, kt, :],
                                     start=(kt == 0), stop=(kt == 7))
            lg4v = lg4.rearrange("p (b e) -> p b e", e=E)

            def bc8(col):  # [128, 4] -> broadcast [128, 4, 8]
                return col.unsqueeze(2).to_broadcast([128, 4, E])

            def bc2(col16):  # [128, 4, 4] -> broadcast [128, 4, 4, 2]
                return col16.unsqueeze(3).to_broadcast([128, 4, G, 2])

            mx4 = wk.tile([128, 4], F32, tag="mx4")
            nc.vector.reduce_max(mx4, lg4v, axis=X)
            sub = wk.tile([128, 4, E], F32, tag="sub")
            nc.vector.tensor_sub(sub, lg4v, bc8(mx4))
            ex = wk.tile([128, 4, E], F32, tag="ex")
            nc.scalar.activation(ex, sub, AF.Exp)
            sm4 = wk.tile([128, 4], F32, tag="sm4")
            nc.vector.reduce_sum(sm4, ex, axis=X)
            rcp4 = wk.tile([128, 4], F32, tag="rcp4")
            nc.vector.reciprocal(rcp4, sm4)
            scores = wk.tile([128, 4, E], F32, tag="scores")
            nc.vector.tensor_mul(scores, ex, bc8(rcp4))
            s = wk.tile([128, 4, E], F32, tag="s")
            nc.vector.tensor_add(s, scores, bias_sb.unsqueeze(1).to_broadcast([128, 4, E]))
            sv = s.rearrange("p b (g two) -> p b g two", two=2)
            g4 = wk.tile([128, 4, G], F32, tag="g4")
            nc.vector.tensor_add(g4, sv[:, :, :, 0], sv[:, :, :, 1])
            gmax = wk.tile([128, 4], F32, tag="gmax")
            nc.vector.reduce_max(gmax, g4, axis=X)
            ohg1 = wk.tile([128, 4, G], F32, tag="ohg1")
            nc.vector.tensor_tensor(ohg1, g4, bc8(gmax)[:, :, 0:G], op=ALU.is_equal)
            g2 = wk.tile([128, 4, G], F32, tag="g2")
            nc.vector.scalar_tensor_tensor(g2, ohg1, -BIG, g4,
                                           op0=ALU.mult, op1=ALU.add)
            gmax2 = wk.tile([128, 4], F32, tag="gmax2")
            nc.vector.reduce_max(gmax2, g2, axis=X)
            ohg2 = wk.tile([128, 4, G], F32, tag="ohg2")
            nc.vector.tensor_tensor(ohg2, g2, bc8(gmax2)[:, :, 0:G], op=ALU.is_equal)
            keep = wk.tile([128, 4, G], F32, tag="keep")
            nc.vector.tensor_add(keep, ohg1, ohg2)
            mk = wk.tile([128, 4, G], F32, tag="mk")
            nc.vector.tensor_scalar(mk, keep, BIG, BIG, op0=ALU.mult, op1=ALU.subtract)
            # masked = s*keep + (keep*BIG - BIG)   (exact select)
            m0 = wk.tile([128, 4, G, 2], F32, tag="m0")
            nc.vector.tensor_mul(m0, sv, bc2(keep))
            masked = wk.tile([128, 4, G, 2], F32, tag="masked")
            nc.vector.tensor_add(masked, m0, bc2(mk))
            maskedv = masked.rearrange("p b g two -> p b (g two)")
            m1 = wk.tile([128, 4], F32, tag="m1")
            nc.vector.reduce_max(m1, maskedv, axis=X)
            # one-hots stored interleaved [128, Jb, k, e]; weights [128, Jb, k]
            ohb = gp.tile([128, 4, 2, E], F32, tag="ohb")
            wtb = gp.tile([128, 4, 2], F32, tag="wtb")
            oh1v = ohb[:, :, 0, :]
            oh2v = ohb[:, :, 1, :]
            nc.vector.tensor_tensor(oh1v, maskedv, bc8(m1), op=ALU.is_equal)
            masked2 = wk.tile([128, 4, E], F32, tag="masked2")
            nc.vector.scalar_tensor_tensor(masked2, oh1v, -BIG, maskedv,
                                           op0=ALU.mult, op1=ALU.add)
            m2 = wk.tile([128, 4], F32, tag="m2")
            nc.vector.reduce_max(m2, masked2, axis=X)
            nc.vector.tensor_tensor(oh2v, masked2, bc8(m2), op=ALU.is_equal)
            tw1 = wk.tile([128, 4, E], F32, tag="tw1")
            nc.vector.tensor_mul(tw1, oh1v, scores)
            nc.vector.reduce_sum(wtb[:, :, 0], tw1, axis=X)
            tw2 = wk.tile([128, 4, E], F32, tag="tw2")
            nc.vector.tensor_mul(tw2, oh2v, scores)
            nc.vector.reduce_sum(wtb[:, :, 1], tw2, axis=X)

            # partial counts, bf16-exact: col j sums 2 row-sets (<=256 each)
            cnt4_ps = pm.tile([E, 4], F32, tag="m2")
            for pi in range(8):
                Jb, k = pi // 2, pi % 2
                nc.tensor.matmul(cnt4_ps[:, ts(pi // 2, 1)],
                                 lhsT=ohb[:, Jb, k, :], rhs=ones_col,
                                 start=(pi % 2 == 0), stop=(pi % 2 == 1))

            # ===== tables for expert e=core; pack [a | b | cnt4] and AllGather =====
            agin_sb = wk.tile([E, 2 * HID + 4], BF16, tag="aginsb")
            nc.scalar.copy(agin_sb[:, 2 * HID:2 * HID + 4], cnt4_ps)
            a_ps = pm.tile([E, HID], F32, tag="m2")
            for kt in range(8):
                nc.tensor.matmul(a_ps, lhsT=x8t_sb[:, kt, :], rhs=w1c_sb[:, kt, :],
                                 start=(kt == 0), stop=(kt == 7))
            nc.scalar.copy(agin_sb[:, 0:HID], a_ps)
            b_ps = pm.tile([E, HID], F32, tag="m2")
            for kt in range(8):
                nc.tensor.matmul(b_ps, lhsT=x8t_sb[:, kt, :], rhs=w3c_sb[:, kt, :],
                                 start=(kt == 0), stop=(kt == 7))
            nc.scalar.copy(agin_sb[:, HID:2 * HID], b_ps)
            nc.scalar.dma_start(agin.ap(), agin_sb)
            nc.gpsimd.collective_compute(
                "AllGather", ALU.bypass, replica_groups=RG,
                ins=[agin.ap().opt()], outs=[agout.ap().opt()],
            )

            # ===== shared-expert FFN (bf16), emitted BEFORE the phi tail: the whole FFN must precede any AG-dependent instruction in every engine queue, because the first collective cannot complete before ~70us (ncfw bootstrap) =====
            hh_sb = wp.tile([128, 8, TC], BF16, tag="hh")
            for J in range(8):
                h_ps = pbig.tile([128, 2 * TC], F32, tag="big")
                for kt in range(8):
                    nc.tensor.matmul(h_ps[:, 0:TC], lhsT=sw1_sb[:, kt, ts(J, 128)],
                                     rhs=xtb_sb[:, kt, :],
                                     start=(kt == 0), stop=(kt == 7))
                for kt in range(8):
                    nc.tensor.matmul(h_ps[:, TC:2 * TC], lhsT=sw3_sb[:, kt, ts(J, 128)],
                                     rhs=xtb_sb[:, kt, :],
                                     start=(kt == 0), stop=(kt == 7))
                sg1 = wk.tile([128, TC], F32, tag="sg1")
                nc.scalar.activation(sg1, h_ps[:, 0:TC], AF.Silu)
                nc.vector.tensor_mul(hh_sb[:, J, :], sg1, h_ps[:, TC:2 * TC])
            # stage2: out[tok, d] with hh stationary -> token-major output
            for tb in range(4):
                o_ps = pbig.tile([128, D], F32, tag="big")
                for J in range(8):
                    nc.tensor.matmul(o_ps[:, 0:512], lhsT=hh_sb[:, J, ts(tb, 128)],
                                     rhs=sw2t_sb[:, J, 0:512],
                                     start=(J == 0), stop=(J == 7))
                for J in range(8):
                    nc.tensor.matmul(o_ps[:, 512:1024], lhsT=hh_sb[:, J, ts(tb, 128)],
                                     rhs=sw2t_sb[:, J, 512:1024],
                                     start=(J == 0), stop=(J == 7))
                o_sb = wk.tile([128, D], BF16, tag="osb")
                nc.vector.tensor_copy(o_sb[:, 0:512], o_ps[:, 0:512])
                nc.scalar.copy(o_sb[:, 512:1024], o_ps[:, 512:1024])
                nc.sync.dma_start(out.ap()[ts(tb, 128), :], o_sb)





            tabs = wp.tile([E * E, 2 * HID + 4], BF16, tag="tabs")
            nc.sync.dma_start(tabs, agout.ap())
            A_bf = tabs[:, 0:HID]
            B_bf = tabs[:, HID:2 * HID]

            # global counts -> -offsets broadcast over 128 partitions
            cnt64 = wk.tile([E * E, 1], F32, tag="cnt64")
            nc.vector.reduce_sum(cnt64, tabs[:, 2 * HID:2 * HID + 4], axis=X)
            rhs64 = wk.tile([E * E, E], F32, tag="rhs64")
            nc.vector.tensor_scalar_mul(rhs64, negLrep_sb, cnt64)
            noffs_ps = pm.tile([128, E], F32, tag="m2")
            nc.tensor.matmul(noffs_ps, lhsT=ones64_sb, rhs=rhs64, start=True, stop=True)
            noffs = wp.tile([128, E], F32, tag="noffs")
            nc.vector.tensor_copy(noffs, noffs_ps)

            # ===== phi phase: batched masks for all 8 row-sets =====
            # Gm8[p, rs, e] = (noffs[p,e] >= niv8[p,rs]) == (global_row + noffs >= 0)
            Gm8 = wk.tile([128, 8, E], F32, tag="Gm8")
            nc.vector.tensor_tensor(
                Gm8,
                noffs.unsqueeze(1).to_broadcast([128, 8, E]),
                niv8_sb.unsqueeze(2).to_broadcast([128, 8, E]),
                op=ALU.is_ge)
            osb8 = wk.tile([128, 8, E], F32, tag="osb8")
            nc.vector.tensor_sub(osb8[:, :, 1:E], Gm8[:, :, 0:E - 1], Gm8[:, :, 1:E])
            nc.vector.tensor_scalar(osb8[:, :, 0:1], Gm8[:, :, 0:1], -1.0, 1.0,
                                    op0=ALU.mult, op1=ALU.add)
            # ote_all[p, rs, e_seg, t_choice]  (0/1 exact, bf16)
            ote_all = gp.tile([128, 8, E, E], BF16, tag="ote_all")
            ohrs = ohb.rearrange("p b k e -> p (b k) e")
            nc.vector.tensor_tensor(
                ote_all,
                osb8.unsqueeze(3).to_broadcast([128, 8, E, E]),
                ohrs.unsqueeze(2).to_broadcast([128, 8, E, E]),
                op=ALU.mult)
            otev = ote_all.rearrange("p r e t -> p r (e t)")
            wtv = wtb.rearrange("p b k -> p (b k)")

            # per-rowset: transpose -> gather a/b -> phi = silu(w*a) * (w*b)
            otT_sb = wp.tile([E * E, 8, 128], BF16, tag="otT")
            phis = []
            for rs in range(8):
                otT_ps = pot.tile([E * E, 128], BF16, tag="otT")
                nc.tensor.transpose(otT_ps, otev[:, rs, :], idbf_sb)
                nc.vector.tensor_copy(otT_sb[:, rs, :], otT_ps)
                ab_ps = pbig.tile([128, 2 * HID], F32, tag="big")
                nc.tensor.matmul(ab_ps[:, 0:HID], lhsT=otT_sb[:, rs, :], rhs=A_bf,
                                 start=True, stop=True)
                nc.tensor.matmul(ab_ps[:, HID:2 * HID], lhsT=otT_sb[:, rs, :], rhs=B_bf,
                                 start=True, stop=True)
                wtk = wtv[:, rs:rs + 1]
                sga = wk.tile([128, HID], F32, tag="sga")
                nc.scalar.activation(sga, ab_ps[:, 0:HID], AF.Silu, scale=wtk)
                phi = gp.tile([128, HID], BF16, tag=f"phi{rs}")
                nc.vector.scalar_tensor_tensor(phi, ab_ps[:, HID:2 * HID], wtk, sga,
                                               op0=ALU.mult, op1=ALU.mult)
                phis.append(phi)

            H_ps = pm.tile([E * E, HID], F32, tag="m1")
            for rs in range(8):
                nc.tensor.matmul(H_ps, lhsT=otev[:, rs, :], rhs=phis[rs],
                                 start=(rs == 0), stop=(rs == 7))
            H_sb = wk.tile([E * E, HID], BF16, tag="Hsb")
            nc.vector.tensor_copy(H_sb, H_ps)
            nc.gpsimd.dma_start(rsin.ap(), H_sb)
            nc.gpsimd.collective_compute(
                "ReduceScatter", ALU.add, replica_groups=RG,
                ins=[rsin.ap().opt()], outs=[rsout.ap().opt()],
            )

            # ===== delta for expert e=core =====
            hc = wk.tile([E, HID], BF16, tag="hc")
            nc.scalar.dma_start(hc, rsout.ap())
            hct = wk.tile([128, 4 * E], BF16, tag="hct")
            hct3 = hct.rearrange("p (q e) -> p q e", q=4)
            for q in range(4):
                tp_ps = pm.tile([128, E], BF16, tag="m2")
                nc.tensor.transpose(tp_ps, hc[:, ts(q, 128)], id8b_sb)
                nc.vector.tensor_copy(hct3[:, q, :], tp_ps)
            for n in range(2):
                d_ps = pm.tile([E, 512], F32, tag="m1")
                for q in range(4):
                    nc.tensor.matmul(d_ps, lhsT=hct3[:, q, :],
                                     rhs=w2c_sb[:, q, ts(n, 512)],
                                     start=(q == 0), stop=(q == 3))
                d_sb = wk.tile([E, 512], F32, tag="dsb")
                nc.scalar.copy(d_sb, d_ps)
                nc.sync.dma_start(dout.ap()[:, ts(n, 512)], d_sb)

    nc.compile()
    return nc


_NC = None


def _get_nc():
    global _NC
    if _NC is None:
        _NC = build()
    return _NC


def _pack(a, k):
    """[k*128, f] -> [128, k, f] partition-major contiguous."""
    kk, f = a.shape
    assert kk == k * 128
    return np.ascontiguousarray(a.reshape(k, 128, f).transpose(1, 0, 2))


def make_in_maps(x, w_gate, w1, w2, w3, sw1, sw2, sw3, expert_bias):
    bf = ml_dtypes.bfloat16
    xf = np.ascontiguousarray(np.asarray(x, np.float32).reshape(NTOK, D))
    x8t_np = _pack(np.ascontiguousarray(xf[:E].T).astype(bf), 8)
    wg_np = _pack(np.ascontiguousarray(np.asarray(w_gate, np.float32).T).astype(bf), 8)
    sw1t_np = _pack(np.ascontiguousarray(np.asarray(sw1, np.float32).T).astype(bf), 8)
    sw3t_np = _pack(np.ascontiguousarray(np.asarray(sw3, np.float32).T).astype(bf), 8)
    sw2t_np = _pack(np.ascontiguousarray(np.asarray(sw2, np.float32).T).astype(bf), 8)
    sw1ta_np = np.ascontiguousarray(sw1t_np[:, :, 0:512])
    sw1tb_np = np.ascontiguousarray(sw1t_np[:, :, 512:1024])
    sw3ta_np = np.ascontiguousarray(sw3t_np[:, :, 0:512])
    sw3tb_np = np.ascontiguousarray(sw3t_np[:, :, 512:1024])
    bias_np = np.ascontiguousarray(np.asarray(expert_bias, np.float32).reshape(1, E))
    w1_np = np.asarray(w1, np.float32)
    w2_np = np.asarray(w2, np.float32)
    w3_np = np.asarray(w3, np.float32)
    # niv8[p, rs] = -(global_row) = -(1024*c + 2*p + 256*(rs//2) + rs%2)
    rsoff = np.array([256 * (r // 2) + (r % 2) for r in range(8)], np.float32)
    p2 = 2.0 * np.arange(128, dtype=np.float32).reshape(128, 1)
    in_maps = []
    for c in range(C):
        xtT = np.ascontiguousarray(xf[c * TC:(c + 1) * TC].T)
        in_maps.append({
            "xtb": _pack(xtT.astype(bf), 8),
            "x8t": x8t_np,
            "wg": wg_np,
            "sw1ta": sw1ta_np,
            "sw1tb": sw1tb_np,
            "sw3ta": sw3ta_np,
            "sw3tb": sw3tb_np,
            "sw2t": sw2t_np,
            "w1c": _pack(np.ascontiguousarray(w1_np[c]).astype(bf), 8),
            "w3c": _pack(np.ascontiguousarray(w3_np[c]).astype(bf), 8),
            "w2c": _pack(np.ascontiguousarray(w2_np[c]).astype(bf), 4),
            "biasd": bias_np,
            "niv8d": np.ascontiguousarray(-(1024.0 * c + p2 + rsoff[None, :])),
        })
    return in_maps


def combine_outputs(results):
    full = np.empty((NTOK, D), np.float32)
    delta = np.zeros((E, D), np.float32)
    for c in range(C):
        full[c * TC:(c + 1) * TC] = results[c]["out"].astype(np.float32)
        delta += results[c]["dout"]
    full[:E] += delta
    return full.reshape(2, 2048, D)


def kernel(x, w_gate, w1, w2, w3, sw1, sw2, sw3, expert_bias, **_unused):
    nc = _get_nc()
    in_maps = make_in_maps(x, w_gate, w1, w2, w3, sw1, sw2, sw3, expert_bias)
    res = bass_utils.run_bass_kernel_spmd(nc, in_maps, core_ids=list(range(C)))
    return combine_outputs(res.results)
